# revision 11
# baseline (speedup 1.0000x reference)
"""CIN (Compressed Interaction Network) kernel for Trainium2, 8 NeuronCores.

Reference computation (per layer k, fused einsum):
    xk_new[b,k,d] = sum_{i,j} W[k, i*n+j] * xk[b,i,d] * x0[b,j,d]
    pooled_k[b,:] = sum_d xk_new[b,:,d]
    out = concat(pooled_1, pooled_2, pooled_3)    # (B, 384)

Mapping:
  - Data-parallel over batch: 8 cores x 128 batches each.
  - On-chip layout: partitions = feature index i (H_prev), free dim =
    columns c = (b_local, d) pairs, processed in chunks of C columns.
  - Per layer, loop j in 0..31:
        Y_j = xk (.) broadcast(x0[:, j, :])    (VectorE tensor_tensor, bf16)
        psum[k, c] += W_j^T @ Y_j              (TensorE, K=H_prev contraction)
    Layer 1 packs 4 j's into one K=128 matmul (H_prev=32) by stacking
    4 partition strips: rhs strip s holds x0[i] * x0[4q+s] products.
  - Pooled sums via VectorE reduce over d-groups; intermediate layers
    round-trip PSUM->SBUF in bf16 via ScalarE copies.
  - Output (k, b) tiles are PE-transposed to (b, k) and DMA'd out.
"""

import os
import sys
from contextlib import ExitStack

sys.path.insert(0, "/opt/trn_rl_repo")
os.environ.setdefault("MYCRO_LOCAL_CACHE", "1")

import numpy as np
import ml_dtypes

import concourse.bass as bass
import concourse.tile as tile
from concourse import bacc, mybir
from concourse.bass_utils import run_bass_kernel_spmd
from concourse.masks import make_identity

B, N, D = 1024, 32, 32
H = 128                     # every layer's output features
NCORES = 8
BC = B // NCORES            # 128 batches per core
COLS = BC * D               # 4096 columns per core
C = 1024                    # chunk columns (32 batches x 32 d)
NB = C // D                 # batches per chunk
NCHUNK = COLS // C
MMN = 512                   # matmul moving free dim (one PSUM bank of fp32)
BF = mybir.dt.bfloat16
F32 = mybir.dt.float32

_CACHE = {}


def _dap(handle, offset, dims):
    a = handle[:]
    return bass.AP(tensor=a.tensor, offset=offset, ap=dims)


def _build_program():
    nc = bacc.Bacc(
        "TRN2", target_bir_lowering=False, debug=False, num_devices=NCORES
    )
    xr = nc.declare_dram_parameter("xr", [128, COLS], BF, isOutput=False)
    f4a = nc.declare_dram_parameter("f4a", [8, 128, COLS], BF, isOutput=False)
    fja = nc.declare_dram_parameter("fja", [N, 128, COLS], BF, isOutput=False)
    w0p = nc.declare_dram_parameter("w0p", [8, 128, H], BF, isOutput=False)
    w1p = nc.declare_dram_parameter("w1p", [N, H, H], BF, isOutput=False)
    w2p = nc.declare_dram_parameter("w2p", [N, H, H], BF, isOutput=False)
    out = nc.declare_dram_parameter("out", [BC, 3 * H], F32, isOutput=True)

    with tile.TileContext(nc) as tc, ExitStack() as ctx:
        singles = ctx.enter_context(tc.tile_pool(name="singles", bufs=1))
        f4pool = ctx.enter_context(tc.tile_pool(name="f4pool", bufs=1))
        fpool = ctx.enter_context(tc.tile_pool(name="fpool", bufs=2))
        x0pool = ctx.enter_context(tc.tile_pool(name="x0pool", bufs=2))
        xpool = ctx.enter_context(tc.tile_pool(name="xpool", bufs=3))
        ypool = ctx.enter_context(tc.tile_pool(name="ypool", bufs=6))
        pspool = ctx.enter_context(tc.tile_pool(name="ps", bufs=3, space="PSUM"))
        tpool = ctx.enter_context(tc.tile_pool(name="tps", bufs=1, space="PSUM"))

        # --- weights, identity, persistent accumulators ---
        w0t = singles.tile([128, 8, H], BF)
        nc.sync.dma_start(out=w0t[:], in_=_dap(w0p, 0, [[H, 128], [128 * H, 8], [1, H]]))
        w1t = singles.tile([128, N, H], BF)
        nc.sync.dma_start(out=w1t[:], in_=_dap(w1p, 0, [[H, 128], [128 * H, N], [1, H]]))
        w2t = singles.tile([128, N, H], BF)
        nc.sync.dma_start(out=w2t[:], in_=_dap(w2p, 0, [[H, 128], [128 * H, N], [1, H]]))
        ident = singles.tile([128, 128], F32)
        make_identity(nc, ident[:])
        pooled = singles.tile([128, 3, BC], F32)
        out_sb = singles.tile([128, 3 * H], F32)

        for ich in range(NCHUNK):
            base = ich * NB * N * D
            bsl = slice(ich * NB, (ich + 1) * NB)

            # x0 replicated: partition p holds row n = p % 32 (host-prepped)
            x0r = x0pool.tile([128, C], BF, tag="x0r")
            nc.scalar.dma_start(
                out=x0r[:],
                in_=_dap(xr, ich * C, [[COLS, 128], [1, C]]),
            )
            # layer-1 factors: f4t[p, q, :] = x0 row (4q + p//32), one DMA
            f4t = f4pool.tile([128, 8, C], BF, tag="f4")
            nc.scalar.dma_start(
                out=f4t[:],
                in_=_dap(f4a, ich * C, [[COLS, 128], [128 * COLS, 8], [1, C]]),
            )
            # layer-2/3 factors: fjt[p, j, :] = x0 row j (all partitions), one DMA
            fjt = fpool.tile([128, N, C], BF, tag="fj")
            for half in range(2):
                hn = N // 2
                nc.sync.dma_start(
                    out=fjt[:, half * hn : (half + 1) * hn, :],
                    in_=_dap(
                        fja,
                        half * hn * 128 * COLS + ich * C,
                        [[COLS, 128], [128 * COLS, hn], [1, C]],
                    ),
                )

            # ---- layer 1: quad-packed, K=128 over (j_local, i) ----
            ps1 = pspool.tile([128, C], F32, tag="ps")
            for q in range(8):
                y = ypool.tile([128, C], BF, tag="y")
                nc.vector.tensor_mul(y[:], x0r[:], f4t[:, q, :])
                for t in range(C // MMN):
                    nc.tensor.matmul(
                        ps1[:, MMN * t : MMN * (t + 1)],
                        lhsT=w0t[:, q, :],
                        rhs=y[:, MMN * t : MMN * (t + 1)],
                        start=(q == 0),
                        stop=(q == 7),
                    )
            x1 = xpool.tile([128, C], BF, tag="x")
            nc.scalar.copy(out=x1[:], in_=ps1[:])
            nc.vector.reduce_sum(
                out=pooled[:, 0, bsl],
                in_=ps1[:].rearrange("p (b d) -> p b d", d=D),
                axis=mybir.AxisListType.X,
            )

            # ---- layer 2 ----
            ps2 = pspool.tile([128, C], F32, tag="ps")
            for j in range(N):
                y = ypool.tile([128, C], BF, tag="y")
                eng = nc.gpsimd if j % 4 == 3 else nc.vector
                eng.tensor_mul(y[:], x1[:], fjt[:, j, :])
                for t in range(C // MMN):
                    nc.tensor.matmul(
                        ps2[:, MMN * t : MMN * (t + 1)],
                        lhsT=w1t[:, j, :],
                        rhs=y[:, MMN * t : MMN * (t + 1)],
                        start=(j == 0),
                        stop=(j == N - 1),
                    )
            x2 = xpool.tile([128, C], BF, tag="x")
            nc.scalar.copy(out=x2[:], in_=ps2[:])
            nc.vector.reduce_sum(
                out=pooled[:, 1, bsl],
                in_=ps2[:].rearrange("p (b d) -> p b d", d=D),
                axis=mybir.AxisListType.X,
            )

            # ---- layer 3 (pooled only) ----
            ps3 = pspool.tile([128, C], F32, tag="ps")
            for j in range(N):
                y = ypool.tile([128, C], BF, tag="y")
                eng = nc.gpsimd if j % 4 == 3 else nc.vector
                eng.tensor_mul(y[:], x2[:], fjt[:, j, :])
                for t in range(C // MMN):
                    nc.tensor.matmul(
                        ps3[:, MMN * t : MMN * (t + 1)],
                        lhsT=w2t[:, j, :],
                        rhs=y[:, MMN * t : MMN * (t + 1)],
                        start=(j == 0),
                        stop=(j == N - 1),
                    )
            nc.vector.reduce_sum(
                out=pooled[:, 2, bsl],
                in_=ps3[:].rearrange("p (b d) -> p b d", d=D),
                axis=mybir.AxisListType.X,
            )

        # ---- finalize: transpose pooled (k, b) -> (b, k), store ----
        for layer in range(3):
            tp = tpool.tile([128, 128], F32, tag="tp")
            nc.tensor.transpose(tp[:], pooled[:, layer, :], ident[:])
            nc.scalar.copy(out=out_sb[:, H * layer : H * (layer + 1)], in_=tp[:])
        nc.sync.dma_start(out=out[:], in_=out_sb[:])

    nc.compile()
    return nc


def _prep_inputs(x0, w0, w1, w2):
    bf = ml_dtypes.bfloat16
    x0b = np.ascontiguousarray(x0.astype(bf))
    # w0: (N*N, H) -> (i, j, k) -> quad-packed (8, 4*32, H), p = jl*32 + i
    w0r = w0.reshape(N, N, H).transpose(1, 0, 2)          # (j, i, k)
    w0q = np.ascontiguousarray(
        w0r.reshape(8, 4, N, H).reshape(8, 128, H).astype(bf)
    )
    w1r = np.ascontiguousarray(
        w1.reshape(H, N, H).transpose(1, 0, 2).astype(bf)  # (j, i, k)
    )
    w2r = np.ascontiguousarray(
        w2.reshape(H, N, H).transpose(1, 0, 2).astype(bf)
    )
    return x0b, w0q, w1r, w2r


def _get_compiled():
    if "nc" not in _CACHE:
        _CACHE["nc"] = _build_program()
    return _CACHE["nc"]


def run(x0, w0, w1, w2, trace=False):
    nc = _get_compiled()
    x0b, w0q, w1r, w2r = _prep_inputs(
        np.asarray(x0, np.float32),
        np.asarray(w0, np.float32),
        np.asarray(w1, np.float32),
        np.asarray(w2, np.float32),
    )
    core_ids = list(range(NCORES))
    quad_rows = np.repeat(np.arange(N), 32).reshape(8, 128)
    in_maps = []
    for c in core_ids:
        shard = np.ascontiguousarray(x0b[c * BC : (c + 1) * BC])
        x0t = np.ascontiguousarray(shard.transpose(1, 0, 2).reshape(N, COLS))
        in_maps.append(
            {
                "xr": np.ascontiguousarray(np.tile(x0t, (4, 1))),
                "f4a": np.ascontiguousarray(x0t[quad_rows]),
                "fja": np.ascontiguousarray(
                    np.broadcast_to(x0t[:, None, :], (N, 128, COLS))
                ),
                "w0p": w0q,
                "w1p": w1r,
                "w2p": w2r,
            }
        )
    res = run_bass_kernel_spmd(nc, in_maps, core_ids, trace=trace)
    outs = [np.asarray(res.results[c]["out"], np.float32) for c in core_ids]
    return np.concatenate(outs, axis=0), res


def kernel(x0, w0, w1, w2):
    full, _ = run(x0, w0, w1, w2, trace=False)
    return full


# revision 12
# speedup vs baseline: 1.3265x; 1.3265x over previous
"""CIN (Compressed Interaction Network) kernel for Trainium2, 8 NeuronCores.

Reference computation (per layer k, fused einsum):
    xk_new[b,k,d] = sum_{i,j} W[k, i*n+j] * xk[b,i,d] * x0[b,j,d]
    pooled_k[b,:] = sum_d xk_new[b,:,d]
    out = concat(pooled_1, pooled_2, pooled_3)    # (B, 384)

Mapping:
  - Data-parallel over batch: 8 cores x 128 batches each.
  - On-chip layout: partitions = feature index i (H_prev), free dim =
    columns c = (b_local, d) pairs, processed in chunks of C columns.
  - Per layer, loop j in 0..31:
        Y_j = xk (.) broadcast(x0[:, j, :])    (VectorE tensor_tensor, bf16)
        psum[k, c] += W_j^T @ Y_j              (TensorE, K=H_prev contraction)
    Layer 1 packs 4 j's into one K=128 matmul (H_prev=32) by stacking
    4 partition strips: rhs strip s holds x0[i] * x0[4q+s] products.
  - Pooled sums via VectorE reduce over d-groups; intermediate layers
    round-trip PSUM->SBUF in bf16 via ScalarE copies.
  - Output (k, b) tiles are PE-transposed to (b, k) and DMA'd out.
"""

import os
import sys
from contextlib import ExitStack

sys.path.insert(0, "/opt/trn_rl_repo")
os.environ.setdefault("MYCRO_LOCAL_CACHE", "1")

import numpy as np
import ml_dtypes

import concourse.bass as bass
import concourse.tile as tile
from concourse import bacc, mybir
from concourse.bass_utils import run_bass_kernel_spmd
from concourse.masks import make_identity

B, N, D = 1024, 32, 32
H = 128                     # every layer's output features
NCORES = 8
BC = B // NCORES            # 128 batches per core
COLS = BC * D               # 4096 columns per core
C = 1024                    # chunk columns (32 batches x 32 d)
NB = C // D                 # batches per chunk
NCHUNK = COLS // C
MMN = 512                   # matmul moving free dim (one PSUM bank of fp32)
BF = mybir.dt.bfloat16
F32 = mybir.dt.float32

_CACHE = {}


def _dap(handle, offset, dims):
    a = handle[:]
    return bass.AP(tensor=a.tensor, offset=offset, ap=dims)


def _build_program():
    nc = bacc.Bacc(
        "TRN2", target_bir_lowering=False, debug=False, num_devices=NCORES
    )
    xr = nc.declare_dram_parameter("xr", [128, COLS], BF, isOutput=False)
    f4a = nc.declare_dram_parameter("f4a", [8, 128, COLS], BF, isOutput=False)
    fja = nc.declare_dram_parameter("fja", [N, 128, COLS], BF, isOutput=False)
    w0p = nc.declare_dram_parameter("w0p", [8, 128, H], BF, isOutput=False)
    w1p = nc.declare_dram_parameter("w1p", [N, H, H], BF, isOutput=False)
    w2p = nc.declare_dram_parameter("w2p", [N, H, H], BF, isOutput=False)
    out = nc.declare_dram_parameter("out", [BC, 3 * H], F32, isOutput=True)

    with tile.TileContext(nc) as tc, ExitStack() as ctx:
        singles = ctx.enter_context(tc.tile_pool(name="singles", bufs=1))
        f4pool = ctx.enter_context(tc.tile_pool(name="f4pool", bufs=1))
        fpool = ctx.enter_context(tc.tile_pool(name="fpool", bufs=1))
        x0pool = ctx.enter_context(tc.tile_pool(name="x0pool", bufs=2))
        xpool = ctx.enter_context(tc.tile_pool(name="xpool", bufs=3))
        ypool = ctx.enter_context(tc.tile_pool(name="ypool", bufs=6))
        pspool = ctx.enter_context(tc.tile_pool(name="ps", bufs=3, space="PSUM"))
        tpool = ctx.enter_context(tc.tile_pool(name="tps", bufs=1, space="PSUM"))

        # --- weights, identity, persistent accumulators ---
        w0t = singles.tile([128, 8, H], BF)
        nc.sync.dma_start(out=w0t[:], in_=_dap(w0p, 0, [[H, 128], [128 * H, 8], [1, H]]))
        w1t = singles.tile([128, N, H], BF)
        nc.sync.dma_start(out=w1t[:], in_=_dap(w1p, 0, [[H, 128], [128 * H, N], [1, H]]))
        w2t = singles.tile([128, N, H], BF)
        nc.sync.dma_start(out=w2t[:], in_=_dap(w2p, 0, [[H, 128], [128 * H, N], [1, H]]))
        ident = singles.tile([128, 128], F32)
        make_identity(nc, ident[:])
        pooled = singles.tile([128, 3, BC], F32)
        out_sb = singles.tile([128, 3 * H], F32)

        def bcast4(tile_ap):
            # (128, C) tile read as (128, 4, C) with the j-dim broadcast
            return bass.AP(
                tensor=tile_ap.tensor,
                offset=tile_ap.offset,
                ap=[tile_ap.ap[0], [0, 4], tile_ap.ap[1]],
            )

        NH = N // 2  # j's per fjt half-tile

        for ich in range(NCHUNK):
            bsl = slice(ich * NB, (ich + 1) * NB)

            # x0 replicated: partition p holds row n = p % 32 (host-prepped)
            x0r = x0pool.tile([128, C], BF, tag="x0r")
            nc.scalar.dma_start(
                out=x0r[:],
                in_=_dap(xr, ich * C, [[COLS, 128], [1, C]]),
            )
            # layer-1 factors: f4t[p, q, :] = x0 row (4q + p//32), one DMA
            f4t = f4pool.tile([128, 8, C], BF, tag="f4")
            nc.scalar.dma_start(
                out=f4t[:],
                in_=_dap(f4a, ich * C, [[COLS, 128], [128 * COLS, 8], [1, C]]),
            )
            # layer-2/3 factors in two half tiles: fj*[p, j, :] = x0 row j
            fjA = fpool.tile([128, NH, C], BF, tag="fjA")
            nc.sync.dma_start(
                out=fjA[:],
                in_=_dap(fja, ich * C, [[COLS, 128], [128 * COLS, NH], [1, C]]),
            )
            fjB = fpool.tile([128, NH, C], BF, tag="fjB")
            nc.sync.dma_start(
                out=fjB[:],
                in_=_dap(
                    fja,
                    NH * 128 * COLS + ich * C,
                    [[COLS, 128], [128 * COLS, NH], [1, C]],
                ),
            )

            def jquad_layer(xk, wt, ps, first_partition=128):
                # one TT per 4 j's: Y[p, jl, c] = xk[p, c] * F[p, j0+jl, c]
                for g in range(8):
                    j0 = 4 * g
                    fh, fo = (fjA, j0) if j0 < NH else (fjB, j0 - NH)
                    y = ypool.tile([128, 4, C], BF, tag="y")
                    nc.vector.tensor_mul(
                        y[:], bcast4(xk[:]), fh[:, fo : fo + 4, :]
                    )
                    for jl in range(4):
                        j = j0 + jl
                        for t in range(C // MMN):
                            nc.tensor.matmul(
                                ps[:, MMN * t : MMN * (t + 1)],
                                lhsT=wt[:, j, :],
                                rhs=y[:, jl, MMN * t : MMN * (t + 1)],
                                start=(j == 0),
                                stop=(j == N - 1),
                            )

            # ---- layer 1: quad-packed, K=128 over (j_local, i) ----
            ps1 = pspool.tile([128, C], F32, tag="ps")
            for g in range(2):
                y = ypool.tile([128, 4, C], BF, tag="y")
                nc.vector.tensor_mul(
                    y[:], bcast4(x0r[:]), f4t[:, 4 * g : 4 * (g + 1), :]
                )
                for ql in range(4):
                    q = 4 * g + ql
                    for t in range(C // MMN):
                        nc.tensor.matmul(
                            ps1[:, MMN * t : MMN * (t + 1)],
                            lhsT=w0t[:, q, :],
                            rhs=y[:, ql, MMN * t : MMN * (t + 1)],
                            start=(q == 0),
                            stop=(q == 7),
                        )
            x1 = xpool.tile([128, C], BF, tag="x")
            nc.scalar.copy(out=x1[:], in_=ps1[:])
            nc.vector.reduce_sum(
                out=pooled[:, 0, bsl],
                in_=ps1[:].rearrange("p (b d) -> p b d", d=D),
                axis=mybir.AxisListType.X,
            )

            # ---- layer 2 ----
            ps2 = pspool.tile([128, C], F32, tag="ps")
            jquad_layer(x1, w1t, ps2)
            x2 = xpool.tile([128, C], BF, tag="x")
            nc.scalar.copy(out=x2[:], in_=ps2[:])
            nc.vector.reduce_sum(
                out=pooled[:, 1, bsl],
                in_=ps2[:].rearrange("p (b d) -> p b d", d=D),
                axis=mybir.AxisListType.X,
            )

            # ---- layer 3 (pooled only) ----
            ps3 = pspool.tile([128, C], F32, tag="ps")
            jquad_layer(x2, w2t, ps3)
            nc.vector.reduce_sum(
                out=pooled[:, 2, bsl],
                in_=ps3[:].rearrange("p (b d) -> p b d", d=D),
                axis=mybir.AxisListType.X,
            )

        # ---- finalize: transpose pooled (k, b) -> (b, k), store ----
        for layer in range(3):
            tp = tpool.tile([128, 128], F32, tag="tp")
            nc.tensor.transpose(tp[:], pooled[:, layer, :], ident[:])
            nc.scalar.copy(out=out_sb[:, H * layer : H * (layer + 1)], in_=tp[:])
        nc.sync.dma_start(out=out[:], in_=out_sb[:])

    nc.compile()
    return nc


def _prep_inputs(x0, w0, w1, w2):
    bf = ml_dtypes.bfloat16
    x0b = np.ascontiguousarray(x0.astype(bf))
    # w0: (N*N, H) -> (i, j, k) -> quad-packed (8, 4*32, H), p = jl*32 + i
    w0r = w0.reshape(N, N, H).transpose(1, 0, 2)          # (j, i, k)
    w0q = np.ascontiguousarray(
        w0r.reshape(8, 4, N, H).reshape(8, 128, H).astype(bf)
    )
    w1r = np.ascontiguousarray(
        w1.reshape(H, N, H).transpose(1, 0, 2).astype(bf)  # (j, i, k)
    )
    w2r = np.ascontiguousarray(
        w2.reshape(H, N, H).transpose(1, 0, 2).astype(bf)
    )
    return x0b, w0q, w1r, w2r


def _get_compiled():
    if "nc" not in _CACHE:
        _CACHE["nc"] = _build_program()
    return _CACHE["nc"]


def run(x0, w0, w1, w2, trace=False):
    nc = _get_compiled()
    x0b, w0q, w1r, w2r = _prep_inputs(
        np.asarray(x0, np.float32),
        np.asarray(w0, np.float32),
        np.asarray(w1, np.float32),
        np.asarray(w2, np.float32),
    )
    core_ids = list(range(NCORES))
    quad_rows = np.repeat(np.arange(N), 32).reshape(8, 128)
    in_maps = []
    for c in core_ids:
        shard = np.ascontiguousarray(x0b[c * BC : (c + 1) * BC])
        x0t = np.ascontiguousarray(shard.transpose(1, 0, 2).reshape(N, COLS))
        in_maps.append(
            {
                "xr": np.ascontiguousarray(np.tile(x0t, (4, 1))),
                "f4a": np.ascontiguousarray(x0t[quad_rows]),
                "fja": np.ascontiguousarray(
                    np.broadcast_to(x0t[:, None, :], (N, 128, COLS))
                ),
                "w0p": w0q,
                "w1p": w1r,
                "w2p": w2r,
            }
        )
    res = run_bass_kernel_spmd(nc, in_maps, core_ids, trace=trace)
    outs = [np.asarray(res.results[c]["out"], np.float32) for c in core_ids]
    return np.concatenate(outs, axis=0), res


def kernel(x0, w0, w1, w2):
    full, _ = run(x0, w0, w1, w2, trace=False)
    return full
